# revision 45
# baseline (speedup 1.0000x reference)
"""Distributed attention kernel for one TRN2 chip (8 NeuronCores).

Problem: multi-head cross-attention
  B=4, TQ=512, TKV=4096, D=1024, H=8 heads (head_dim=128)

Sharding (data-parallel x tensor-parallel, per the hint):
  core c in 0..7 -> (batch b = c % 4, head-group g = c // 4)
  Each core computes heads [4g, 4g+4) for its batch: Wq/Wk/Wv column
  shards, Wo row shard. Each core DMAs its [P, 8, TQ] head-group
  partial of the output projection to DRAM and the host sums the
  (c, c+4) pairs (a device ReduceScatter measured ~17us of serial
  tail; the host add is free).

Fully *streamed* device schedule: after the Q projection, the kernel
loops over the 8 KV T-chunks; for each chunk it interleaves the K/V
projection matmuls of chunk c+1 with the attention units of chunk c
(one unit = one (T-block, head): S matmul -> exp -> mask -> U/den
accumulate). The ACT-engine exp (~720ns/unit) therefore hides under
the much larger projection matmul stream instead of pacing a separate
attention phase.

Per-unit device math (everything transposed so no on-device
transposes; the host passes x^T and mask^T):
  Q^T[dh, t]  = Wq_g^T x_q^T (+bq)     K^T[dh, T] = Wk_g^T x_kv^T (+bk)
  V[T, dh]    = x_kv Wv_g (+bv)
  S^T[T, t]   = K^T_h(block)^T Q^T_h   per (block, head)
  P^T         = exp(S^T/sqrt(128)) * mask^T  (no max-subtraction:
                scores are O(1) so exp cannot overflow/underflow)
  U^T[dh, t] += V_h(block)^T P^T       accumulated in PSUM (4 banks)
  den_h[t]   += ones[P,32]^T P^T       col-strip matmul into partition
                group [32h, 32h+32) of ONE psum bank; the 4 strips of
                a T-block group run concurrently in the PE array
  ut = U * 1/den  (approx reciprocal), out^T[o, t] = Wo_g^T ut (+bo on
  group 0 only), partials DMAed out as they finish.

Matmul inputs are bf16 (PE 2x faster than fp32); PSUM accumulation and
softmax denominators stay fp32.
"""

import sys

if "/opt/trn_rl_repo" not in sys.path:
    sys.path.insert(0, "/opt/trn_rl_repo")

import numpy as np
import ml_dtypes
from contextlib import ExitStack

B, TQ, TKV, D, H = 4, 512, 4096, 1024, 8
HD = D // H            # 128 head dim
NCORES = 8
GH = H // 2            # heads per core = 4
GD = GH * HD           # 512 cols per head-group
P = 128
KC = D // P            # 8 contraction chunks
NTB = TKV // P         # 32 T-blocks
NTC = TKV // 512       # 8 T-chunks (DMA granularity)
SCALE = float(1.0 / np.sqrt(HD))
NU = NTC * 4 * GH      # 128 attention units: (chunk, block, head)

_CACHED_NC = None


def _build_nc():
    from concourse import mybir, bacc
    from concourse.tile import TileContext

    bf = mybir.dt.bfloat16
    f32 = mybir.dt.float32
    AF = mybir.ActivationFunctionType
    OP = mybir.AluOpType

    nc = bacc.Bacc("TRN2", target_bir_lowering=False, debug=False,
                   num_devices=NCORES)

    # All inputs are pre-tiled on the host into partition-major layouts
    # so every DMA is 128 contiguous multi-KB descriptors.
    f8 = mybir.dt.float8e4
    xqT = nc.dram_tensor("xqT", [P, KC, TQ], bf, kind="ExternalInput")
    xkvT = nc.dram_tensor("xkvT", [P, NTC, KC, 512], bf, kind="ExternalInput")
    # fp8 copy of x_kv^T rows [0,512) for the K projection (DoubleRow
    # layout [K=128, ktile, pair, T]); host pre-scales by 8 to clear the
    # e4m3 subnormal range. Only HALF the K contraction runs in fp8:
    # full-fp8 measured rel err 1.87e-2 against the 2e-2 gate; half
    # keeps it ~1.3e-2 while still saving ~13us of PE time. The bf16
    # half is host-prescaled by 512 so both partials share one exact
    # 1/512 post-scale.
    xkv8T = nc.dram_tensor("xkv8T", [P, NTC, KC // 4, 2, 512], f8,
                           kind="ExternalInput")
    maskT = nc.dram_tensor("maskT", [P, NTB, TQ], bf, kind="ExternalInput")
    Wq = nc.dram_tensor("Wq", [P, KC, GD], bf, kind="ExternalInput")
    # Wk rows [0,512) in fp8 DoubleRow layout (x64); rows [512,1024) in
    # bf16 (x512, exact power-of-2 scale)
    Wk = nc.dram_tensor("Wk", [P, KC // 4, 2, GD], f8, kind="ExternalInput")
    WkB = nc.dram_tensor("WkB", [P, KC // 2, GD], bf, kind="ExternalInput")
    Wv = nc.dram_tensor("Wv", [P, KC, GD], bf, kind="ExternalInput")
    Wo = nc.dram_tensor("Wo", [P, GH, D], bf, kind="ExternalInput")
    bq = nc.dram_tensor("bq", [GD], f32, kind="ExternalInput")
    bk = nc.dram_tensor("bk", [GD], f32, kind="ExternalInput")
    bv = nc.dram_tensor("bv", [GD], f32, kind="ExternalInput")
    bo = nc.dram_tensor("bo", [D], f32, kind="ExternalInput")
    out = nc.dram_tensor("out", [P, D // P, TQ], bf, kind="ExternalOutput")

    with TileContext(nc) as tc:
        with ExitStack() as ctx:
            persist = ctx.enter_context(tc.tile_pool(name="persist", bufs=1))
            kvchunk = ctx.enter_context(tc.tile_pool(name="kvchunk", bufs=3))
            kvproj = ctx.enter_context(tc.tile_pool(name="kvproj", bufs=2))
            work = ctx.enter_context(tc.tile_pool(name="work", bufs=3))
            outp = ctx.enter_context(tc.tile_pool(name="outp", bufs=2))
            # PSUM budget (8 banks): ppool 3 x [P,TQ] rotating (proj
            # tiles, S tiles, warm-up, out-proj) + upool 1 x [P,4,TQ]
            # (U accumulators, one bank per head) + dpool 1 x [P,TQ]
            # (den, one 32-partition strip group per head).
            ppool = ctx.enter_context(
                tc.tile_pool(name="ppool", bufs=3, space="PSUM"))
            upool = ctx.enter_context(
                tc.tile_pool(name="upool", bufs=1, space="PSUM"))
            dpool = ctx.enter_context(
                tc.tile_pool(name="dpool", bufs=1, space="PSUM"))

            # ---- DMA queue order == emission order ---------------------
            # The 16 HW queues drain a shared FIFO prefix: a tile is
            # usable when everything emitted before it has landed
            # (~0.43 MB/us after a ~10us ramp). Order by first-use time.
            # K-projection data first (smallest prefix that unlocks real
            # PE work: 0.5MB of fp8 starts the chunk-0 K DR matmuls),
            # then the Q operands, then V.
            wk_sb = persist.tile([P, KC // 4, 2, GD], f8)
            wkb_sb = persist.tile([P, KC // 2, GD], bf)
            wv_sb = persist.tile([P, KC, GD], bf)
            kv_tiles, kv8_tiles = {}, {}

            def load_kv_chunk(tcknk):
                # fp8 (K-proj) first: the K steps run before the V steps
                t8 = kvchunk.tile([P, KC // 4, 2, 512], f8, name="xkv8_t",
                                  tag="xkv8")
                nc.sync.dma_start(t8[:], xkv8T.ap()[:, tcknk, :, :, :])
                kv8_tiles[tcknk] = t8
                t = kvchunk.tile([P, KC, 512], bf, name="xkv_t", tag="xkv")
                nc.sync.dma_start(t[:], xkvT.ap()[:, tcknk, :, :])
                kv_tiles[tcknk] = t

            nc.sync.dma_start(wk_sb[:], Wk.ap())
            nc.sync.dma_start(wkb_sb[:], WkB.ap())
            load_kv_chunk(0)
            wq_sb = persist.tile([P, KC, GD], bf)
            xq_sb = persist.tile([P, KC, TQ], bf)
            nc.sync.dma_start(wq_sb[:], Wq.ap())
            nc.sync.dma_start(xq_sb[:], xqT.ap())
            # tiny bias descriptors (128 x 16B each) ride between the
            # big tiles; needed from ~20us (K/Q bias) onward
            bq_sb = persist.tile([P, GH], f32)
            bk_sb = persist.tile([P, GH], f32)
            nc.sync.dma_start(bq_sb[:], bq.ap().rearrange("(h p) -> p h", p=P))
            nc.sync.dma_start(bk_sb[:], bk.ap().rearrange("(h p) -> p h", p=P))
            bv_row = persist.tile([1, GD], f32)
            nc.sync.dma_start(bv_row[:], bv.ap().unsqueeze(0))
            nc.sync.dma_start(wv_sb[:], Wv.ap())
            load_kv_chunk(1)
            mask_sb = persist.tile([P, NTB, TQ], bf)

            def load_mask_chunk(c):
                nc.sync.dma_start(mask_sb[:, 4 * c:4 * c + 4, :],
                                  maskT.ap()[:, 4 * c:4 * c + 4, :])

            load_mask_chunk(0)  # chunk 0 mask needed ~30us in
            load_mask_chunk(1)

            bv_rep = persist.tile([P, GD], f32)
            nc.gpsimd.partition_broadcast(bv_rep[:], bv_row[:])

            # ---- constants --------------------------------------------
            ones_bf = persist.tile([P, P], bf)
            nc.vector.memset(ones_bf[:], 1.0)
            # 1e-32 fill: rhs for PE warm-up matmuls and the den epsilon
            # seed (128 * 1e-32 floor keeps 1/den finite; all-masked
            # rows then give ut = 0 exactly, matching the wipe).
            eps_sb = persist.tile([P, TQ], bf)
            nc.vector.memset(eps_sb[:], 1e-32)
            # c32: [32, 128] of 1/32 -> c32^T @ den32 averages the 32
            # identical replicated rows of a den strip group and
            # broadcasts the result across all 128 output partitions.
            c32 = persist.tile([32, P], bf)
            nc.vector.memset(c32[:], 1.0 / 32.0)

            # PE warm-up: dummy matmuls from t~0 keep the PE busy until
            # the first weights land (~11us) so the HAM clock gate is at
            # 2.4 GHz when real work starts.
            warm_ps = ppool.tile([P, TQ], f32, name="warm", tag="ps")
            for _ in range(32):
                nc.tensor.matmul(warm_ps[:], ones_bf[:], eps_sb[:],
                                 start=True, stop=True)

            # ---- persistent attention state ---------------------------
            u_ps = upool.tile([P, GH, TQ], f32, name="u_ps")
            den_ps = dpool.tile([P, TQ], f32, name="den_ps")
            # epsilon seed; start=True sets has_written for the whole
            # bank so all den strip matmuls accumulate with start=False
            nc.tensor.matmul(den_ps[:], ones_bf[:], eps_sb[:],
                             start=True, stop=False, skip_group_check=True)

            ut_sb = persist.tile([P, GH, TQ], bf)
            kt_bufs, v_bufs = {}, {}

            def proj_steps(c):
                """8 emission closures: K dbs then V tbs for chunk c."""
                kt_t = kvproj.tile([P, GH, 512], bf, name="kt_t", tag="kt")
                v_t = kvproj.tile([P, 4, GD], bf, name="v_t", tag="vt")
                kt_bufs[c], v_bufs[c] = kt_t, v_t
                xkv_t = kv_tiles.pop(c)
                xkv8_t = kv8_tiles.pop(c)

                def k_step(db):
                    # K rows [0,512) via fp8 DoubleRow (256-row passes),
                    # rows [512,1024) in bf16; both partials carry a
                    # 512x power-of-2 pre-scale undone in the bias move.
                    ps = ppool.tile([P, TQ], f32, name="k_ps", tag="ps")
                    for k2 in range(KC // 4):
                        nc.tensor.matmul(ps[:],
                                         wk_sb[:, k2, :, db * P:(db + 1) * P],
                                         xkv8_t[:, k2, :, :],
                                         start=(k2 == 0), stop=False,
                                         perf_mode=mybir.MatmulPerfMode.DoubleRow,
                                         skip_group_check=True)
                    for kc in range(KC // 2):
                        nc.tensor.matmul(ps[:],
                                         wkb_sb[:, kc, db * P:(db + 1) * P],
                                         xkv_t[:, KC // 2 + kc, :],
                                         start=False, stop=(kc == KC // 2 - 1),
                                         skip_group_check=True)
                    nc.vector.scalar_tensor_tensor(
                        kt_t[:, db, :], ps[:], 1.0 / 512.0,
                        bk_sb[:, db:db + 1].to_broadcast([P, 512]),
                        OP.mult, OP.add)

                def v_step(tb):
                    ps = ppool.tile([P, TQ], f32, name="v_ps", tag="ps")
                    for kc in range(KC):
                        nc.tensor.matmul(ps[:],
                                         xkv_t[:, kc, tb * P:(tb + 1) * P],
                                         wv_sb[:, kc, :],
                                         start=(kc == 0), stop=(kc == KC - 1))
                    nc.vector.tensor_tensor(v_t[:, tb, :], ps[:], bv_rep[:],
                                            OP.add)

                return ([lambda db=db: k_step(db) for db in range(GH)]
                        + [lambda tb=tb: v_step(tb) for tb in range(4)])

            # ---- attention unit pipeline ------------------------------
            # unit g = (chunk c, block jb, head h), h-innermost. Slot g
            # emits: S(g+2) [PE], exp+mask(g+1) [ACT/DVE], U(g) [PE],
            # and after h==3 the 4 concurrent den strips of the block.
            s_tiles, p_tiles = {}, {}

            def unit(g):
                c = g // 16
                if c == NTC - 1:
                    # last chunk runs head-outer so each head's den
                    # completes early and its normalize chain inlines
                    # into the (ACT-paced, PE-slack) unit stream
                    return c, g % 4, (g % 16) // 4
                return c, (g % 16) // 4, g % 4  # c, jb, h

            def emit_S(g):
                c, jb, h = unit(g)
                s = ppool.tile([P, TQ], f32, name="s_ps", tag="ps")
                nc.tensor.matmul(s[:],
                                 kt_bufs[c][:, h, jb * P:(jb + 1) * P],
                                 qt_sb[:, h, :], start=True, stop=True)
                s_tiles[g] = s

            def emit_pm(g):
                c, jb, h = unit(g)
                praw = work.tile([P, TQ], bf, tag="praw", bufs=3)
                nc.scalar.activation(praw[:], s_tiles.pop(g)[:], AF.Exp,
                                     scale=SCALE)
                p_t = work.tile([P, TQ], bf, tag="p_t", bufs=8)
                nc.vector.tensor_tensor(p_t[:], praw[:],
                                        mask_sb[:, 4 * c + jb, :], OP.mult)
                p_tiles[g] = p_t

            def emit_chain(h):
                # ut = U / den for head h; den strip group h is 32
                # identical rows, c32 averages + broadcasts them to 128
                den32 = work.tile([32, TQ], bf, tag="den32")
                nc.scalar.copy(den32[:], den_ps[32 * h:32 * h + 32, :])
                rep_ps = ppool.tile([P, TQ], f32, name="rep_ps", tag="ps")
                nc.tensor.matmul(rep_ps[:], c32[:], den32[:],
                                 start=True, stop=True)
                recip = work.tile([P, TQ], f32, tag="recip")
                nc.vector.reciprocal_approx_fast(recip[:], rep_ps[:])
                nc.vector.tensor_tensor(ut_sb[:, h, :], u_ps[:, h, :],
                                        recip[:], OP.mult)

            def emit_U(g):
                c, jb, h = unit(g)
                j = 4 * c + jb
                nc.tensor.matmul(u_ps[:, h, :],
                                 v_bufs[c][:, jb, h * P:(h + 1) * P],
                                 p_tiles[g][:],
                                 start=(j == 0), stop=(j == NTB - 1),
                                 skip_group_check=True)
                # den col-strip for this unit into partition group
                # [32h, 32h+32) of the single den bank (full-width
                # per-head den would need 4 banks the budget lacks)
                nc.tensor.matmul(
                    den_ps[32 * h:32 * h + 32, :],
                    ones_bf[:, 0:32], p_tiles[g][:],
                    start=False, stop=(j == NTB - 1),
                    tile_position=(0, 32 * h),
                    skip_group_check=True)
                p_tiles.pop(g)
                if c == NTC - 1 and jb == 3:
                    emit_chain(h)

            def slot(g):
                if g + 2 < NU:
                    emit_S(g + 2)
                if g + 1 < NU:
                    emit_pm(g + 1)
                emit_U(g)

            # ---- streamed main loop -----------------------------------
            # chunk-0 K steps first (their fp8 data is the first DMA to
            # land), then the Q projection, then chunk-0 V steps
            psteps = proj_steps(0)
            for st in psteps[:GH]:
                st()
            qt_sb = persist.tile([P, GH, TQ], bf)
            for db in range(GH):
                ps = ppool.tile([P, TQ], f32, name="q_ps", tag="ps")
                for kc in range(KC):
                    nc.tensor.matmul(ps[:], wq_sb[:, kc, db * P:(db + 1) * P],
                                     xq_sb[:, kc, :],
                                     start=(kc == 0), stop=(kc == KC - 1))
                nc.vector.tensor_tensor(
                    qt_sb[:, db, :], ps[:],
                    bq_sb[:, db:db + 1].to_broadcast([P, TQ]), OP.add)
            for st in psteps[GH:]:
                st()
            emit_S(0)
            emit_S(1)
            emit_pm(0)
            g = 0
            for c in range(NTC):
                if c + 1 < NTC:
                    if c + 2 < NTC:
                        load_kv_chunk(c + 2)
                        load_mask_chunk(c + 2)
                    if c == 4:
                        # out-proj weights + bias, needed ~30us later
                        wo_sb = persist.tile([P, GH, D], bf)
                        bo_sb = persist.tile([P, D // P], f32)
                        nc.sync.dma_start(wo_sb[:], Wo.ap())
                        nc.sync.dma_start(
                            bo_sb[:], bo.ap().rearrange("(ob p) -> p ob", p=P))
                    psteps = proj_steps(c + 1)
                    for i in range(8):
                        psteps[i]()
                        slot(g)
                        g += 1
                        slot(g)
                        g += 1
                else:
                    while g < NU:
                        slot(g)
                        g += 1

            # ---- out^T partial = Wo_g^T ut (+bo on group 0) -----------
            # Software-pipelined over the 3 psum bufs: emit hc0-2 of
            # blocks b, b+1, b+2 before any block's final hc3 matmul, so
            # the PE has ut3-independent work while the h=3 normalize
            # chain (recip etc.) drains on DVE.
            NOB = D // P
            o_halves = [outp.tile([P, NOB // 2, TQ], bf, name="o_half",
                                  tag="o_half") for _ in range(2)]
            o_ps = {}

            def o_head(b):
                ps = ppool.tile([P, TQ], f32, name="o_ps", tag="ps")
                for hc in range(GH - 1):
                    nc.tensor.matmul(ps[:], wo_sb[:, hc, b * P:(b + 1) * P],
                                     ut_sb[:, hc, :],
                                     start=(hc == 0), stop=False,
                                     skip_group_check=True)
                o_ps[b] = ps

            o_head(0)
            o_head(1)
            for ob in range(NOB):
                if ob + 2 < NOB:
                    o_head(ob + 2)
                ps = o_ps.pop(ob)
                nc.tensor.matmul(ps[:], wo_sb[:, GH - 1, ob * P:(ob + 1) * P],
                                 ut_sb[:, GH - 1, :], start=False, stop=True,
                                 skip_group_check=True)
                half, oi = divmod(ob, NOB // 2)
                nc.vector.tensor_tensor(
                    o_halves[half][:, oi, :], ps[:],
                    bo_sb[:, ob:ob + 1].to_broadcast([P, TQ]), OP.add)
                if ob >= NOB - 2:  # last two blocks: DMA singly (tail)
                    nc.sync.dma_start(out.ap()[:, ob:ob + 1, :],
                                      o_halves[half][:, oi:oi + 1, :])
                elif oi % 2 == 1:  # stream out every 2 o-blocks
                    nc.sync.dma_start(
                        out.ap()[:, ob - 1:ob + 1, :],
                        o_halves[half][:, oi - 1:oi + 1, :])

    nc.finalize()
    return nc


def _shard_inputs(inputs_q, inputs_kv, attention_mask, Wq, bq, Wk, bk, Wv, bv,
                  Wo, bo):
    bf16 = ml_dtypes.bfloat16
    f32 = np.float32

    def ptile(a2d, inner):
        """[R, C] row-major -> [P, R//P, C] partition-major, contiguous."""
        r, c = a2d.shape
        return np.ascontiguousarray(
            a2d.reshape(r // P, P, c).transpose(1, 0, 2)).astype(inner)

    fp8 = ml_dtypes.float8_e4m3

    in_maps = []
    xqT = [ptile(inputs_q[b].T, bf16) for b in range(B)]          # [P,KC,TQ]
    xkvT = [ptile(inputs_kv[b].T, bf16)                           # [P,NTC,KC,512]
            .reshape(P, KC, NTC, 512).transpose(0, 2, 1, 3).copy()
            for b in range(B)]
    # fp8 DoubleRow copy of rows [0,512) for the K projection: x*8
    # clears the e4m3 subnormal range; pair kc = 2*k2 + i on dim "i"
    xkv8T = [(ptile(inputs_kv[b].T * 8.0, fp8)[:, 0:KC // 2]
              .reshape(P, KC // 4, 2, NTC, 512).transpose(0, 3, 1, 2, 4)
              .copy()) for b in range(B)]
    maskT = [ptile(attention_mask[b].T.astype(np.float32), bf16)  # [P,NTB,TQ]
             for b in range(B)]
    for c in range(NCORES):
        b, g = c % B, c // B  # pair = (b, b+4)
        sl = slice(g * GD, (g + 1) * GD)
        in_maps.append({
            "xqT": xqT[b],
            "xkvT": xkvT[b],
            "xkv8T": xkv8T[b],
            "maskT": maskT[b],
            "Wq": ptile(np.ascontiguousarray(Wq[:, sl]), bf16),
            "Wk": (ptile(np.ascontiguousarray(Wk[:, sl]) * 64.0, fp8)
                   [:, 0:KC // 2].reshape(P, KC // 4, 2, GD).copy()),
            "WkB": (ptile(np.ascontiguousarray(Wk[:, sl]) * 512.0, bf16)
                    [:, KC // 2:].copy()),
            "Wv": ptile(np.ascontiguousarray(Wv[:, sl]), bf16),
            "Wo": ptile(np.ascontiguousarray(Wo[sl, :]), bf16),
            "bq": np.ascontiguousarray(bq[sl]).astype(f32),
            "bk": np.ascontiguousarray(bk[sl]).astype(f32),
            "bv": np.ascontiguousarray(bv[sl]).astype(f32),
            "bo": (bo.astype(f32) if g == 0 else np.zeros(D, f32)),
        })
    return in_maps


def kernel(_trace=False, **inputs):
    global _CACHED_NC
    from concourse import bass_utils

    arrs = {k: np.asarray(v) for k, v in inputs.items()}
    in_maps = _shard_inputs(**arrs)

    if _CACHED_NC is None:
        _CACHED_NC = _build_nc()

    res = bass_utils.run_bass_kernel_spmd(
        _CACHED_NC, in_maps, core_ids=list(range(NCORES)), trace=_trace)

    full = np.empty((B, TQ, D), np.float32)
    for b in range(B):
        # host pair-sum of the two head-group partials [P, NOB, TQ]
        psum = (res.results[b]["out"].astype(np.float32)
                + res.results[b + 4]["out"].astype(np.float32))
        outT = psum.transpose(1, 0, 2).reshape(D, TQ)  # [o, t]
        full[b] = outT.T
    if _trace:
        return full, res
    return full


# revision 46
# speedup vs baseline: 1.1057x; 1.1057x over previous
"""Distributed attention kernel for one TRN2 chip (8 NeuronCores).

Problem: multi-head cross-attention
  B=4, TQ=512, TKV=4096, D=1024, H=8 heads (head_dim=128)

Sharding (data-parallel x tensor-parallel, per the hint):
  core c in 0..7 -> (batch b = c % 4, head-group g = c // 4)
  Each core computes heads [4g, 4g+4) for its batch: Wq/Wk/Wv column
  shards, Wo row shard. Each core DMAs its [P, 8, TQ] head-group
  partial of the output projection to DRAM and the host sums the
  (c, c+4) pairs (a device ReduceScatter measured ~17us of serial
  tail; the host add is free).

Fully *streamed* device schedule: after the Q projection, the kernel
loops over the 8 KV T-chunks; for each chunk it interleaves the K/V
projection matmuls of chunk c+1 with the attention units of chunk c
(one unit = one (T-block, head): S matmul -> exp -> mask -> U/den
accumulate). The ACT-engine exp (~720ns/unit) therefore hides under
the much larger projection matmul stream instead of pacing a separate
attention phase.

Per-unit device math (everything transposed so no on-device
transposes; the host passes x^T and mask^T):
  Q^T[dh, t]  = Wq_g^T x_q^T (+bq)     K^T[dh, T] = Wk_g^T x_kv^T (+bk)
  V[T, dh]    = x_kv Wv_g (+bv)
  S^T[T, t]   = K^T_h(block)^T Q^T_h   per (block, head)
  P^T         = exp(S^T/sqrt(128)) * mask^T  (no max-subtraction:
                scores are O(1) so exp cannot overflow/underflow)
  U^T[dh, t] += V_h(block)^T P^T       accumulated in PSUM (4 banks)
  den_h[t]   += ones[P,32]^T P^T       col-strip matmul into partition
                group [32h, 32h+32) of ONE psum bank; the 4 strips of
                a T-block group run concurrently in the PE array
  ut = U * 1/den  (approx reciprocal), out^T[o, t] = Wo_g^T ut (+bo on
  group 0 only), partials DMAed out as they finish.

Matmul inputs are bf16 (PE 2x faster than fp32); PSUM accumulation and
softmax denominators stay fp32.
"""

import sys

if "/opt/trn_rl_repo" not in sys.path:
    sys.path.insert(0, "/opt/trn_rl_repo")

import numpy as np
import ml_dtypes
from contextlib import ExitStack

B, TQ, TKV, D, H = 4, 512, 4096, 1024, 8
HD = D // H            # 128 head dim
NCORES = 8
GH = H // 2            # heads per core = 4
GD = GH * HD           # 512 cols per head-group
P = 128
KC = D // P            # 8 contraction chunks
NTB = TKV // P         # 32 T-blocks
NTC = TKV // 512       # 8 T-chunks (DMA granularity)
SCALE = float(1.0 / np.sqrt(HD))
NU = NTC * 4 * GH      # 128 attention units: (chunk, block, head)

_CACHED_NC = None


def _build_nc():
    from concourse import mybir, bacc
    from concourse.tile import TileContext

    bf = mybir.dt.bfloat16
    f32 = mybir.dt.float32
    AF = mybir.ActivationFunctionType
    OP = mybir.AluOpType

    nc = bacc.Bacc("TRN2", target_bir_lowering=False, debug=False,
                   num_devices=NCORES)

    # All inputs are pre-tiled on the host into partition-major layouts
    # so every DMA is 128 contiguous multi-KB descriptors.
    f8 = mybir.dt.float8e4
    xqT = nc.dram_tensor("xqT", [P, KC, TQ], bf, kind="ExternalInput")
    xkvT = nc.dram_tensor("xkvT", [P, NTC, KC, 512], bf, kind="ExternalInput")
    # fp8 copy of x_kv^T rows [0,512) for the K projection (DoubleRow
    # layout [K=128, ktile, pair, T]); host pre-scales by 8 to clear the
    # e4m3 subnormal range. Only HALF the K contraction runs in fp8:
    # full-fp8 measured rel err 1.87e-2 against the 2e-2 gate; half
    # keeps it ~1.3e-2 while still saving ~13us of PE time. The bf16
    # half is host-prescaled by 512 so both partials share one exact
    # 1/512 post-scale.
    xkv8T = nc.dram_tensor("xkv8T", [P, NTC, KC // 4, 2, 512], f8,
                           kind="ExternalInput")
    maskT = nc.dram_tensor("maskT", [P, NTB, TQ], bf, kind="ExternalInput")
    Wq = nc.dram_tensor("Wq", [P, KC, GD], bf, kind="ExternalInput")
    # Wk rows [0,512) in fp8 DoubleRow layout (x64); rows [512,1024) in
    # bf16 (x512, exact power-of-2 scale)
    Wk = nc.dram_tensor("Wk", [P, KC // 4, 2, GD], f8, kind="ExternalInput")
    WkB = nc.dram_tensor("WkB", [P, KC // 2, GD], bf, kind="ExternalInput")
    Wv = nc.dram_tensor("Wv", [P, KC, GD], bf, kind="ExternalInput")
    Wo = nc.dram_tensor("Wo", [P, GH, D], bf, kind="ExternalInput")
    bq = nc.dram_tensor("bq", [GD], f32, kind="ExternalInput")
    bk = nc.dram_tensor("bk", [GD], f32, kind="ExternalInput")
    bv = nc.dram_tensor("bv", [GD], f32, kind="ExternalInput")
    bo = nc.dram_tensor("bo", [D], f32, kind="ExternalInput")
    out = nc.dram_tensor("out", [P, D // P, TQ], bf, kind="ExternalOutput")

    with TileContext(nc) as tc:
        with ExitStack() as ctx:
            persist = ctx.enter_context(tc.tile_pool(name="persist", bufs=1))
            kvchunk = ctx.enter_context(tc.tile_pool(name="kvchunk", bufs=3))
            kvproj = ctx.enter_context(tc.tile_pool(name="kvproj", bufs=2))
            work = ctx.enter_context(tc.tile_pool(name="work", bufs=3))
            outp = ctx.enter_context(tc.tile_pool(name="outp", bufs=2))
            # PSUM budget (8 banks): ppool 3 x [P,TQ] rotating (proj
            # tiles, S tiles, warm-up, out-proj) + upool 1 x [P,4,TQ]
            # (U accumulators, one bank per head) + dpool 1 x [P,TQ]
            # (den, one 32-partition strip group per head).
            ppool = ctx.enter_context(
                tc.tile_pool(name="ppool", bufs=3, space="PSUM"))
            upool = ctx.enter_context(
                tc.tile_pool(name="upool", bufs=1, space="PSUM"))
            dpool = ctx.enter_context(
                tc.tile_pool(name="dpool", bufs=1, space="PSUM"))

            # ---- DMA queue order == emission order ---------------------
            # The 16 HW queues drain a shared FIFO prefix: a tile is
            # usable when everything emitted before it has landed
            # (~0.43 MB/us after a ~10us ramp). Order by first-use time.
            # K-projection data first (smallest prefix that unlocks real
            # PE work: 0.5MB of fp8 starts the chunk-0 K DR matmuls),
            # then the Q operands, then V.
            wk_sb = persist.tile([P, KC // 4, 2, GD], f8)
            wkb_sb = persist.tile([P, KC // 2, GD], bf)
            wv_sb = persist.tile([P, KC, GD], bf)
            kv_tiles, kv8_tiles = {}, {}

            def load_kv_chunk(tcknk):
                # fp8 (K-proj) first: the K steps run before the V steps
                t8 = kvchunk.tile([P, KC // 4, 2, 512], f8, name="xkv8_t",
                                  tag="xkv8")
                nc.sync.dma_start(t8[:], xkv8T.ap()[:, tcknk, :, :, :])
                kv8_tiles[tcknk] = t8
                t = kvchunk.tile([P, KC, 512], bf, name="xkv_t", tag="xkv")
                nc.sync.dma_start(t[:], xkvT.ap()[:, tcknk, :, :])
                kv_tiles[tcknk] = t

            nc.sync.dma_start(wk_sb[:], Wk.ap())
            nc.sync.dma_start(wkb_sb[:], WkB.ap())
            load_kv_chunk(0)
            wq_sb = persist.tile([P, KC, GD], bf)
            xq_sb = persist.tile([P, KC, TQ], bf)
            nc.sync.dma_start(wq_sb[:], Wq.ap())
            nc.sync.dma_start(xq_sb[:], xqT.ap())
            # tiny bias descriptors (128 x 16B each) ride between the
            # big tiles; needed from ~20us (K/Q bias) onward
            bq_sb = persist.tile([P, GH], f32)
            bk_sb = persist.tile([P, GH], f32)
            nc.sync.dma_start(bq_sb[:], bq.ap().rearrange("(h p) -> p h", p=P))
            nc.sync.dma_start(bk_sb[:], bk.ap().rearrange("(h p) -> p h", p=P))
            bv_row = persist.tile([1, GD], f32)
            nc.sync.dma_start(bv_row[:], bv.ap().unsqueeze(0))
            nc.sync.dma_start(wv_sb[:], Wv.ap())
            load_kv_chunk(1)
            mask_sb = persist.tile([P, NTB, TQ], bf)

            def load_mask_chunk(c):
                nc.sync.dma_start(mask_sb[:, 4 * c:4 * c + 4, :],
                                  maskT.ap()[:, 4 * c:4 * c + 4, :])

            load_mask_chunk(0)  # chunk 0 mask needed ~30us in
            load_mask_chunk(1)

            bv_rep = persist.tile([P, GD], f32)
            nc.gpsimd.partition_broadcast(bv_rep[:], bv_row[:])

            # ---- constants --------------------------------------------
            ones_bf = persist.tile([P, P], bf)
            nc.vector.memset(ones_bf[:], 1.0)
            # 1e-32 fill: rhs for PE warm-up matmuls and the den epsilon
            # seed (128 * 1e-32 floor keeps 1/den finite; all-masked
            # rows then give ut = 0 exactly, matching the wipe).
            eps_sb = persist.tile([P, TQ], bf)
            nc.vector.memset(eps_sb[:], 1e-32)
            # c32: [32, 128] of 1/32 -> c32^T @ den32 averages the 32
            # identical replicated rows of a den strip group and
            # broadcasts the result across all 128 output partitions.
            c32 = persist.tile([32, P], bf)
            nc.vector.memset(c32[:], 1.0 / 32.0)

            # PE warm-up: dummy matmuls from t~0 keep the PE busy until
            # the first weights land (~11us) so the HAM clock gate is at
            # 2.4 GHz when real work starts.
            warm_ps = ppool.tile([P, TQ], f32, name="warm", tag="ps")
            for _ in range(32):
                nc.tensor.matmul(warm_ps[:], ones_bf[:], eps_sb[:],
                                 start=True, stop=True)

            # ---- persistent attention state ---------------------------
            u_ps = upool.tile([P, GH, TQ], f32, name="u_ps")
            den_ps = dpool.tile([P, TQ], f32, name="den_ps")
            # epsilon seed; start=True sets has_written for the whole
            # bank so all den strip matmuls accumulate with start=False
            nc.tensor.matmul(den_ps[:], ones_bf[:], eps_sb[:],
                             start=True, stop=False, skip_group_check=True)

            ut_sb = persist.tile([P, GH, TQ], bf)
            kt_bufs, v_bufs = {}, {}

            def proj_steps(c):
                """8 emission closures: K dbs then V tbs for chunk c."""
                kt_t = kvproj.tile([P, GH, 512], bf, name="kt_t", tag="kt")
                v_t = kvproj.tile([P, 4, GD], bf, name="v_t", tag="vt")
                kt_bufs[c], v_bufs[c] = kt_t, v_t
                xkv_t = kv_tiles.pop(c)
                xkv8_t = kv8_tiles.pop(c)

                def k_step(db):
                    # K rows [0,512) via fp8 DoubleRow (256-row passes),
                    # rows [512,1024) in bf16; both partials carry a
                    # 512x power-of-2 pre-scale undone in the bias move.
                    ps = ppool.tile([P, TQ], f32, name="k_ps", tag="ps")
                    for k2 in range(KC // 4):
                        nc.tensor.matmul(ps[:],
                                         wk_sb[:, k2, :, db * P:(db + 1) * P],
                                         xkv8_t[:, k2, :, :],
                                         start=(k2 == 0), stop=False,
                                         perf_mode=mybir.MatmulPerfMode.DoubleRow,
                                         skip_group_check=True)
                    for kc in range(KC // 2):
                        nc.tensor.matmul(ps[:],
                                         wkb_sb[:, kc, db * P:(db + 1) * P],
                                         xkv_t[:, KC // 2 + kc, :],
                                         start=False, stop=(kc == KC // 2 - 1),
                                         skip_group_check=True)
                    nc.vector.scalar_tensor_tensor(
                        kt_t[:, db, :], ps[:], 1.0 / 512.0,
                        bk_sb[:, db:db + 1].to_broadcast([P, 512]),
                        OP.mult, OP.add)

                def v_step(tb):
                    ps = ppool.tile([P, TQ], f32, name="v_ps", tag="ps")
                    for kc in range(KC):
                        nc.tensor.matmul(ps[:],
                                         xkv_t[:, kc, tb * P:(tb + 1) * P],
                                         wv_sb[:, kc, :],
                                         start=(kc == 0), stop=(kc == KC - 1))
                    nc.vector.tensor_tensor(v_t[:, tb, :], ps[:], bv_rep[:],
                                            OP.add)

                return ([lambda db=db: k_step(db) for db in range(GH)]
                        + [lambda tb=tb: v_step(tb) for tb in range(4)])

            # ---- attention unit pipeline ------------------------------
            # unit g = (chunk c, block jb, head h), h-innermost. Slot g
            # emits: S(g+2) [PE], exp+mask(g+1) [ACT/DVE], U(g) [PE],
            # and after h==3 the 4 concurrent den strips of the block.
            s_tiles, p_tiles = {}, {}

            def unit(g):
                c = g // 16
                if c == NTC - 1:
                    # last chunk runs head-outer so each head's den
                    # completes early and its normalize chain inlines
                    # into the (ACT-paced, PE-slack) unit stream
                    return c, g % 4, (g % 16) // 4
                return c, (g % 16) // 4, g % 4  # c, jb, h

            def emit_S(g):
                c, jb, h = unit(g)
                s = ppool.tile([P, TQ], f32, name="s_ps", tag="ps")
                nc.tensor.matmul(s[:],
                                 kt_bufs[c][:, h, jb * P:(jb + 1) * P],
                                 qt_sb[:, h, :], start=True, stop=True)
                s_tiles[g] = s

            def emit_pm(g):
                c, jb, h = unit(g)
                praw = work.tile([P, TQ], bf, tag="praw", bufs=3)
                nc.scalar.activation(praw[:], s_tiles.pop(g)[:], AF.Exp,
                                     scale=SCALE)
                p_t = work.tile([P, TQ], bf, tag="p_t", bufs=8)
                nc.vector.tensor_tensor(p_t[:], praw[:],
                                        mask_sb[:, 4 * c + jb, :], OP.mult)
                p_tiles[g] = p_t

            def emit_chain(h):
                # ut = U / den for head h; den strip group h is 32
                # identical rows, c32 averages + broadcasts them to 128
                den32 = work.tile([32, TQ], bf, tag="den32")
                nc.scalar.copy(den32[:], den_ps[32 * h:32 * h + 32, :])
                rep_ps = ppool.tile([P, TQ], f32, name="rep_ps", tag="ps")
                nc.tensor.matmul(rep_ps[:], c32[:], den32[:],
                                 start=True, stop=True)
                recip = work.tile([P, TQ], f32, tag="recip")
                nc.vector.reciprocal_approx_fast(recip[:], rep_ps[:])
                nc.vector.tensor_tensor(ut_sb[:, h, :], u_ps[:, h, :],
                                        recip[:], OP.mult)

            def emit_U(g):
                # den strips go in groups of 4 consecutive matmuls: each
                # full-array<->strip transition costs ~130ns on the PE,
                # so interleaving a strip after every U is ~25us slower.
                c, jb, h = unit(g)
                j = 4 * c + jb
                nc.tensor.matmul(u_ps[:, h, :],
                                 v_bufs[c][:, jb, h * P:(h + 1) * P],
                                 p_tiles[g][:],
                                 start=(j == 0), stop=(j == NTB - 1),
                                 skip_group_check=True)
                if c < NTC - 1:
                    if h == GH - 1:
                        # strips for the 4 heads of this T-block
                        for hh in range(GH):
                            nc.tensor.matmul(
                                den_ps[32 * hh:32 * hh + 32, :],
                                ones_bf[:, 0:32], p_tiles[g - 3 + hh][:],
                                start=False, stop=False,
                                tile_position=(0, 32 * hh),
                                skip_group_check=True)
                        for hh in range(GH):
                            p_tiles.pop(g - 3 + hh)
                elif jb == 3:
                    # head-outer last chunk: strips for this head's 4
                    # blocks (same array position), then its chain
                    for k in range(4):
                        nc.tensor.matmul(
                            den_ps[32 * h:32 * h + 32, :],
                            ones_bf[:, 0:32], p_tiles[g - 3 + k][:],
                            start=False, stop=(k == 3),
                            tile_position=(0, 32 * h),
                            skip_group_check=True)
                    for k in range(4):
                        p_tiles.pop(g - 3 + k)
                    emit_chain(h)

            def slot(g):
                if g + 2 < NU:
                    emit_S(g + 2)
                if g + 1 < NU:
                    emit_pm(g + 1)
                emit_U(g)

            # ---- streamed main loop -----------------------------------
            # chunk-0 K steps first (their fp8 data is the first DMA to
            # land), then the Q projection, then chunk-0 V steps
            psteps = proj_steps(0)
            for st in psteps[:GH]:
                st()
            qt_sb = persist.tile([P, GH, TQ], bf)
            for db in range(GH):
                ps = ppool.tile([P, TQ], f32, name="q_ps", tag="ps")
                for kc in range(KC):
                    nc.tensor.matmul(ps[:], wq_sb[:, kc, db * P:(db + 1) * P],
                                     xq_sb[:, kc, :],
                                     start=(kc == 0), stop=(kc == KC - 1))
                nc.vector.tensor_tensor(
                    qt_sb[:, db, :], ps[:],
                    bq_sb[:, db:db + 1].to_broadcast([P, TQ]), OP.add)
            for st in psteps[GH:]:
                st()
            emit_S(0)
            emit_S(1)
            emit_pm(0)
            g = 0
            for c in range(NTC):
                if c + 1 < NTC:
                    if c + 2 < NTC:
                        load_kv_chunk(c + 2)
                        load_mask_chunk(c + 2)
                    if c == 4:
                        # out-proj weights + bias, needed ~30us later
                        wo_sb = persist.tile([P, GH, D], bf)
                        bo_sb = persist.tile([P, D // P], f32)
                        nc.sync.dma_start(wo_sb[:], Wo.ap())
                        nc.sync.dma_start(
                            bo_sb[:], bo.ap().rearrange("(ob p) -> p ob", p=P))
                    psteps = proj_steps(c + 1)
                    for i in range(8):
                        psteps[i]()
                        slot(g)
                        g += 1
                        slot(g)
                        g += 1
                else:
                    while g < NU:
                        slot(g)
                        g += 1

            # ---- out^T partial = Wo_g^T ut (+bo on group 0) -----------
            # Software-pipelined over the 3 psum bufs: emit hc0-2 of
            # blocks b, b+1, b+2 before any block's final hc3 matmul, so
            # the PE has ut3-independent work while the h=3 normalize
            # chain (recip etc.) drains on DVE.
            NOB = D // P
            o_halves = [outp.tile([P, NOB // 2, TQ], bf, name="o_half",
                                  tag="o_half") for _ in range(2)]
            o_ps = {}

            def o_head(b):
                ps = ppool.tile([P, TQ], f32, name="o_ps", tag="ps")
                for hc in range(GH - 1):
                    nc.tensor.matmul(ps[:], wo_sb[:, hc, b * P:(b + 1) * P],
                                     ut_sb[:, hc, :],
                                     start=(hc == 0), stop=False,
                                     skip_group_check=True)
                o_ps[b] = ps

            o_head(0)
            o_head(1)
            for ob in range(NOB):
                if ob + 2 < NOB:
                    o_head(ob + 2)
                ps = o_ps.pop(ob)
                nc.tensor.matmul(ps[:], wo_sb[:, GH - 1, ob * P:(ob + 1) * P],
                                 ut_sb[:, GH - 1, :], start=False, stop=True,
                                 skip_group_check=True)
                half, oi = divmod(ob, NOB // 2)
                nc.vector.tensor_tensor(
                    o_halves[half][:, oi, :], ps[:],
                    bo_sb[:, ob:ob + 1].to_broadcast([P, TQ]), OP.add)
                if ob >= NOB - 2:  # last two blocks: DMA singly (tail)
                    nc.sync.dma_start(out.ap()[:, ob:ob + 1, :],
                                      o_halves[half][:, oi:oi + 1, :])
                elif oi % 2 == 1:  # stream out every 2 o-blocks
                    nc.sync.dma_start(
                        out.ap()[:, ob - 1:ob + 1, :],
                        o_halves[half][:, oi - 1:oi + 1, :])

    nc.finalize()
    return nc


def _shard_inputs(inputs_q, inputs_kv, attention_mask, Wq, bq, Wk, bk, Wv, bv,
                  Wo, bo):
    bf16 = ml_dtypes.bfloat16
    f32 = np.float32

    def ptile(a2d, inner):
        """[R, C] row-major -> [P, R//P, C] partition-major, contiguous."""
        r, c = a2d.shape
        return np.ascontiguousarray(
            a2d.reshape(r // P, P, c).transpose(1, 0, 2)).astype(inner)

    fp8 = ml_dtypes.float8_e4m3

    in_maps = []
    xqT = [ptile(inputs_q[b].T, bf16) for b in range(B)]          # [P,KC,TQ]
    xkvT = [ptile(inputs_kv[b].T, bf16)                           # [P,NTC,KC,512]
            .reshape(P, KC, NTC, 512).transpose(0, 2, 1, 3).copy()
            for b in range(B)]
    # fp8 DoubleRow copy of rows [0,512) for the K projection: x*8
    # clears the e4m3 subnormal range; pair kc = 2*k2 + i on dim "i"
    xkv8T = [(ptile(inputs_kv[b].T * 8.0, fp8)[:, 0:KC // 2]
              .reshape(P, KC // 4, 2, NTC, 512).transpose(0, 3, 1, 2, 4)
              .copy()) for b in range(B)]
    maskT = [ptile(attention_mask[b].T.astype(np.float32), bf16)  # [P,NTB,TQ]
             for b in range(B)]
    for c in range(NCORES):
        b, g = c % B, c // B  # pair = (b, b+4)
        sl = slice(g * GD, (g + 1) * GD)
        in_maps.append({
            "xqT": xqT[b],
            "xkvT": xkvT[b],
            "xkv8T": xkv8T[b],
            "maskT": maskT[b],
            "Wq": ptile(np.ascontiguousarray(Wq[:, sl]), bf16),
            "Wk": (ptile(np.ascontiguousarray(Wk[:, sl]) * 64.0, fp8)
                   [:, 0:KC // 2].reshape(P, KC // 4, 2, GD).copy()),
            "WkB": (ptile(np.ascontiguousarray(Wk[:, sl]) * 512.0, bf16)
                    [:, KC // 2:].copy()),
            "Wv": ptile(np.ascontiguousarray(Wv[:, sl]), bf16),
            "Wo": ptile(np.ascontiguousarray(Wo[sl, :]), bf16),
            "bq": np.ascontiguousarray(bq[sl]).astype(f32),
            "bk": np.ascontiguousarray(bk[sl]).astype(f32),
            "bv": np.ascontiguousarray(bv[sl]).astype(f32),
            "bo": (bo.astype(f32) if g == 0 else np.zeros(D, f32)),
        })
    return in_maps


def kernel(_trace=False, **inputs):
    global _CACHED_NC
    from concourse import bass_utils

    arrs = {k: np.asarray(v) for k, v in inputs.items()}
    in_maps = _shard_inputs(**arrs)

    if _CACHED_NC is None:
        _CACHED_NC = _build_nc()

    res = bass_utils.run_bass_kernel_spmd(
        _CACHED_NC, in_maps, core_ids=list(range(NCORES)), trace=_trace)

    full = np.empty((B, TQ, D), np.float32)
    for b in range(B):
        # host pair-sum of the two head-group partials [P, NOB, TQ]
        psum = (res.results[b]["out"].astype(np.float32)
                + res.results[b + 4]["out"].astype(np.float32))
        outT = psum.transpose(1, 0, 2).reshape(D, TQ)  # [o, t]
        full[b] = outT.T
    if _trace:
        return full, res
    return full
